# revision 1
# baseline (speedup 1.0000x reference)
"""Expert-parallel MoE conditional feed-forward for 8 Trainium2 NeuronCores.

Problem: x[16,1024], expert_indices[16,2], gate/down_proj[8,2816,1024],
up_proj[8,1024,2816]. Reference computes, per (token, slot) pair with
e = expert_indices[t, a]:
    out[t,a,:] = (silu(x @ gate_proj[e].T) * (x @ down_proj[e].T)) @ up_proj[e].T

Sharding: core k owns expert k and computes its FFN output for ALL 16
tokens (the compute is negligible; the kernel is weight-streaming bound).
The host then gathers rows per expert_indices. This needs no indices on
device and is load-balanced regardless of routing.

Weights and x are cast to fp16 on the host (kernel is weight-streaming
bound; the harness gate is 2e-2 and fp16 end-to-end measures 4.7e-4,
while fp8 e4m3 measures 2.7e-2+ even on a single matrix — mantissa-
limited, so 2 B/elem is the floor). This halves HBM traffic to 17.3 MB
per core (~41 us at the ~425 GB/s per-core fair share measured when all
8 cores stream) and takes fp16 matmuls to 1 cycle/column, so the PE
(~28 us of moving columns) is no longer the critical path.

Device kernel (per core): loop over 11 chunks of 256 of the 2816-wide
intermediate dim. Weights are host-packed into W[11, 128, 6144] fp16:
    W[c,p, hc*512+o]        o<256: gate block g[c*256+o, hc*128+p]
                            o>=256: down block d[c*256+o-256, hc*128+p]
    W[c,p, 4096+f*1024+j]   up block u[j, c*256+f*128+p]
All big matmuls stream the WEIGHT as the moving operand (the stationary
is a 16-column token tile). The whole fp16 weight set stays resident in
SBUF (11 tiles, 132 KB/partition) with ONE DMA descriptor per chunk:
descriptor issue is ~650 ns of serial Sync-engine time, so descriptor
count — not DGE bandwidth — sets the stream ramp (each big DMA fans out
across all 16 DGEs on its own). Chunk 0 is split in three so the PE
starts sooner.

PE scheduling (array packing via PSUM base partition; q3 unusable per
HW bug):
    q1 (psum rows 32-47): gate|down chains for even chunks
    q2 (psum rows 64-79): gate|down chains for odd chunks
    q0 (psum rows  0-15): all up-projection accumulation into psum_out
Chunks are processed in pairs; the previous pair's 8 up-matmuls are
round-robin interleaved with the current pair's 2x8 chain matmuls so
consecutive PE instructions hit different strips and overlap. The
[16,128] fp16 intermediates are transposed to [128,16] via identity
matmuls on the PE, cast to fp16 on the PSUM->SBUF copy, and fed as
stationaries. This matters because the PE p-state ramps down when idle
(mid p-state is ~1.35 GHz vs 2.4 max), and the chip duty-cycles the PE
under power throttle — so minimizing PE instruction time keeps the
after-stream tail at ~3 us.

Measured: 63.5-65 us median HW exec (vs 110-124 us fp32 baseline);
frob rel err 4.7e-4.
"""

import sys

for _p in ("/opt/trn_rl_repo", "/opt/pypackages"):
    if _p not in sys.path:
        sys.path.append(_p)

import numpy as np

NUM_EXPERTS = 8
HIDDEN = 1024
INTER = 2816
T = 16
N_CORES = 8
P = 128
CW = 256                  # intermediate chunk width
NCHUNK = INTER // CW      # 11
HC = HIDDEN // P          # 8 hidden chunks
U_OFF = 2 * HC * CW       # 4096: offset of up blocks in packed W
WCOLS = U_OFF + 2 * HIDDEN  # 6144

_COMPILED = None
LAST_RESULTS = None
TRACE = False


def _build():
    import concourse.bacc as bacc
    import concourse.bass as bass
    import concourse.tile as tile
    from concourse import mybir

    f32 = mybir.dt.float32
    f16 = mybir.dt.float16
    nc = bacc.Bacc("TRN2", target_bir_lowering=False, debug=False,
                   num_devices=N_CORES)
    xt_d = nc.dram_tensor("xt", [P, HC * T], f16, kind="ExternalInput")
    eye_d = nc.dram_tensor("eye", [T, T], f16, kind="ExternalInput")
    w_d = nc.dram_tensor("w", [NCHUNK, P, WCOLS], f16, kind="ExternalInput")
    out_d = nc.dram_tensor("out", [T, HIDDEN], f32, kind="ExternalOutput")

    with tile.TileContext(nc) as tc:
        with (
            tc.tile_pool(name="xp", bufs=1) as xp,
            tc.tile_pool(name="wp", bufs=1) as wp,
            tc.tile_pool(name="ip", bufs=4) as ip,
            tc.tile_pool(name="itp", bufs=1) as itp,
            tc.tile_pool(name="pg", bufs=4, space=bass.MemorySpace.PSUM) as pgp,
            tc.tile_pool(name="tp", bufs=2, space=bass.MemorySpace.PSUM) as tpp,
            tc.tile_pool(name="po", bufs=1, space=bass.MemorySpace.PSUM) as pop,
            tc.tile_pool(name="op", bufs=1) as op,
        ):
            xt = xp.tile([P, HC * T], f16)
            nc.sync.dma_start(xt[:], xt_d.ap())
            eye = xp.tile([T, T], f16)
            nc.sync.dma_start(eye[:], eye_d.ap())

            # Whole fp16 weight set resident in SBUF (132 KB/partition),
            # ONE descriptor per chunk: the Sync engine issues descriptors
            # serially at ~650 ns each, so descriptor count — not DGE
            # bandwidth — sets the stream ramp. Each big DMA fans out
            # across all 16 DGEs on its own. Chunk 0 is split finer so the
            # PE starts sooner.
            wt = []
            for c in range(NCHUNK):
                t = wp.tile([P, WCOLS], f16, tag=f"w{c}")
                if c == 0:
                    nc.sync.dma_start(t[:, 0:U_OFF // 2],
                                      w_d.ap()[c][:, 0:U_OFF // 2])
                    nc.sync.dma_start(t[:, U_OFF // 2:U_OFF],
                                      w_d.ap()[c][:, U_OFF // 2:U_OFF])
                    nc.sync.dma_start(t[:, U_OFF:WCOLS],
                                      w_d.ap()[c][:, U_OFF:WCOLS])
                else:
                    nc.sync.dma_start(t[:], w_d.ap()[c])
                wt.append(t)

            psum_out = pop.tile([T, HIDDEN], f32)
            itall = itp.tile([P, 2 * NCHUNK * T], f16)
            mm3_count = [0, 0]   # per-jb position in the accumulation chain
            pending_mm3 = []     # thunks deferred from the previous pair

            def emit_chunk_tail(c, pgd, base):
                """silu/mul + transposes for chunk c; queue its 4 up-matmuls."""
                s1 = ip.tile([T, CW], f32)
                nc.scalar.activation(s1[:], pgd[base:base + T, 0:CW],
                                     mybir.ActivationFunctionType.Silu)
                inter = ip.tile([T, CW], f16)
                nc.vector.tensor_mul(inter[:], s1[:],
                                     pgd[base:base + T, CW:2 * CW])
                for f in range(CW // P):
                    tp = tpp.tile([P, T], f32)
                    nc.tensor.matmul(tp[:], inter[:, f * P:(f + 1) * P], eye[:])
                    k = 2 * c + f
                    it = itall[:, k * T:(k + 1) * T]
                    nc.vector.tensor_copy(it, tp[:])
                    for jb in range(HIDDEN // 512):
                        def mm3(it=it, c=c, f=f, jb=jb):
                            k = mm3_count[jb]
                            mm3_count[jb] += 1
                            nc.tensor.matmul(
                                psum_out[:, jb * 512:(jb + 1) * 512], it,
                                wt[c][:, U_OFF + f * HIDDEN + jb * 512:
                                      U_OFF + f * HIDDEN + (jb + 1) * 512],
                                start=(k == 0), stop=(k == 2 * NCHUNK - 1),
                            )
                        pending_mm3.append(mm3)

            for c0 in range(0, NCHUNK, 2):
                pair = [c0] + ([c0 + 1] if c0 + 1 < NCHUNK else [])
                tiles = []
                for c, base in zip(pair, (32, 64)):
                    pgd = pgp.tile([P, 2 * CW], f32)
                    tiles.append((c, pgd, base))

                todo = pending_mm3
                pending_mm3 = []
                for hc in range(HC):
                    for c, pgd, base in tiles:
                        nc.tensor.matmul(
                            pgd[base:base + T, :], xt[:, hc * T:(hc + 1) * T],
                            wt[c][:, hc * 2 * CW:(hc + 1) * 2 * CW],
                            start=(hc == 0), stop=(hc == HC - 1),
                        )
                    if todo:
                        todo.pop(0)()
                while todo:
                    todo.pop(0)()

                for c, pgd, base in tiles:
                    emit_chunk_tail(c, pgd, base)

            for mm3 in pending_mm3:
                mm3()

            out_sb = op.tile([T, HIDDEN], f32)
            for jb in range(HIDDEN // 512):
                nc.vector.tensor_copy(out_sb[:, jb * 512:(jb + 1) * 512],
                                      psum_out[:, jb * 512:(jb + 1) * 512])
            nc.sync.dma_start(out_d.ap(), out_sb[:])

    nc.compile()
    return nc


def _get_compiled():
    global _COMPILED
    if _COMPILED is None:
        _COMPILED = _build()
    return _COMPILED


def _pack_inputs(x, gate_proj, up_proj, down_proj):
    x = np.ascontiguousarray(x, dtype=np.float32)
    # xt[p, hc*T + t] = x[t, hc*128 + p]
    xt = np.ascontiguousarray(
        x.T.reshape(HC, P, T).transpose(1, 0, 2).reshape(P, HC * T)
    ).astype(np.float16)
    eye = np.eye(T, dtype=np.float16)
    in_maps = []
    for k in range(N_CORES):
        g = np.asarray(gate_proj[k], dtype=np.float32)
        d = np.asarray(down_proj[k], dtype=np.float32)
        u = np.asarray(up_proj[k], dtype=np.float32)
        # wg4/wd4[c, p, hc, o] = g/d[c*CW + o, hc*128 + p]; interleave per hc
        wg4 = g.reshape(NCHUNK, CW, HC, P).transpose(0, 3, 2, 1)
        wd4 = d.reshape(NCHUNK, CW, HC, P).transpose(0, 3, 2, 1)
        wgd = np.concatenate([wg4, wd4], axis=3).reshape(NCHUNK, P, 2 * HC * CW)
        # Wu[c, p, f*HIDDEN + j] = u[j, c*CW + f*128 + p]
        wu = u.reshape(HIDDEN, NCHUNK, CW // P, P).transpose(1, 3, 2, 0).reshape(
            NCHUNK, P, 2 * HIDDEN)
        w = np.ascontiguousarray(
            np.concatenate([wgd, wu], axis=2), dtype=np.float16)
        in_maps.append({"xt": xt, "eye": eye, "w": w})
    return in_maps


def kernel(x, expert_indices, gate_proj, up_proj, down_proj):
    global LAST_RESULTS
    from concourse.bass_utils import run_bass_kernel_spmd

    nc = _get_compiled()
    in_maps = _pack_inputs(x, gate_proj, up_proj, down_proj)
    res = run_bass_kernel_spmd(nc, in_maps, core_ids=list(range(N_CORES)),
                               trace=TRACE)
    LAST_RESULTS = res

    expert_outs = np.stack([res.results[k]["out"] for k in range(N_CORES)])
    idx = np.asarray(expert_indices).astype(np.int64)  # [T, TOP_K]
    return expert_outs[idx, np.arange(T)[:, None], :].astype(np.float32)



# revision 3
# speedup vs baseline: 1.1319x; 1.1319x over previous
"""Expert-parallel MoE conditional feed-forward for 8 Trainium2 NeuronCores.

Problem: x[16,1024], expert_indices[16,2], gate/down_proj[8,2816,1024],
up_proj[8,1024,2816]. Reference computes, per (token, slot) pair with
e = expert_indices[t, a]:
    out[t,a,:] = (silu(x @ gate_proj[e].T) * (x @ down_proj[e].T)) @ up_proj[e].T

Sharding: core k owns expert k and computes its FFN output for ALL 16
tokens (the compute is negligible; the kernel is weight-streaming bound).
The host then gathers rows per expert_indices. This needs no indices on
device and is load-balanced regardless of routing.

Weights and x are cast to fp16 on the host (harness gate is 2e-2; fp16
end-to-end measures 4.7e-4 while fp8 e4m3 is mantissa-limited at 2.7e-2+
per matrix). 17.3 MB per core streams at the ~420 GB/s per-core fabric
ceiling measured when all 8 cores stream (~42 us).

Timeline anatomy (measured): exec_time = [first GpSimd MEMSET ... last
epilogue NOTIFY]. A ~5.8 us start rendezvous is excluded; a fixed
~8.4 us walrus epilogue (per-semaphore zeroing avalanche) is included.
So the controllable part is first-DMA-issue -> out-DMA-complete.

Key structural choices vs the previous revision:
  * Weight chunk 0's first DMA is issued BEFORE xt/eye so streaming (the
    critical path) starts ~1.3 us earlier.
  * Chunks are processed singly (not in pairs): chunk c's 4 up-matmuls
    are deferred only into chunk c+1's gate/down chain. That keeps the
    end-of-stream backlog to one chunk.
  * Chunks 0, 9, 10 stream as split DMAs; Tile's range-level dependency
    tracking lets consuming matmuls start as soon as their slice lands.
    Chunk 10 additionally uses a per-half column layout and is processed
    as two independent 128-wide half-chunks, so the dependency chain
    behind the very last weight byte is one 8-matmul N=256 chain + one
    silu/mul/transpose + 2 up-matmuls.
  * The final PSUM->SBUF copies run in parallel (jb0 on DVE, jb1 on the
    Scalar engine as an activation-Copy) before a single output DMA.

PE scheduling (array packing via PSUM base partition; q3 unusable per
HW bug): q1 (psum rows 32-47) and q2 (rows 64-79) alternate per chunk
for the gate|down chains; q0 (rows 0-15) carries all up-projection
accumulation into psum_out. The [16,128] fp16 intermediates are
transposed to [128,16] via identity matmuls on the PE, cast to fp16 on
the PSUM->SBUF copy, and fed as stationaries.
"""

import sys

for _p in ("/opt/trn_rl_repo", "/opt/pypackages"):
    if _p not in sys.path:
        sys.path.append(_p)

import numpy as np

NUM_EXPERTS = 8
HIDDEN = 1024
INTER = 2816
T = 16
N_CORES = 8
P = 128
CW = 256                  # intermediate chunk width
NCHUNK = INTER // CW      # 11
NFULL = NCHUNK - 1        # chunks 0..9 use the full-chunk layout
HC = HIDDEN // P          # 8 hidden chunks
U_OFF = 2 * HC * CW       # 4096: offset of up blocks in packed W
WCOLS = U_OFF + 2 * HIDDEN  # 6144
N_UP = 2 * NFULL + 2      # 22 up-matmuls per 512-col output half

_COMPILED = None
LAST_RESULTS = None
TRACE = False


def _build():
    import concourse.bacc as bacc
    import concourse.bass as bass
    import concourse.tile as tile
    from concourse import mybir

    f32 = mybir.dt.float32
    f16 = mybir.dt.float16
    nc = bacc.Bacc("TRN2", target_bir_lowering=False, debug=False,
                   num_devices=N_CORES)
    xt_d = nc.dram_tensor("xt", [P, HC * T], f16, kind="ExternalInput")
    eye_d = nc.dram_tensor("eye", [T, T], f16, kind="ExternalInput")
    w_d = nc.dram_tensor("w", [NCHUNK, P, WCOLS], f16, kind="ExternalInput")
    out_d = nc.dram_tensor("out", [T, HIDDEN], f32, kind="ExternalOutput")

    with tile.TileContext(nc) as tc:
        with (
            tc.tile_pool(name="xp", bufs=1) as xp,
            tc.tile_pool(name="wp", bufs=1) as wp,
            tc.tile_pool(name="ip", bufs=4) as ip,
            tc.tile_pool(name="itp", bufs=1) as itp,
            tc.tile_pool(name="pg", bufs=3, space=bass.MemorySpace.PSUM) as pgp,
            tc.tile_pool(name="tp", bufs=2, space=bass.MemorySpace.PSUM) as tpp,
            tc.tile_pool(name="po", bufs=1, space=bass.MemorySpace.PSUM) as pop,
            tc.tile_pool(name="op", bufs=1) as op,
        ):
            xt = xp.tile([P, HC * T], f16)
            eye = xp.tile([T, T], f16)
            wt = [wp.tile([P, WCOLS], f16, name=f"w{c}", tag=f"w{c}")
                  for c in range(NCHUNK)]

            # DMA issue order (all on Sync/HWDGE; descriptor generation is
            # ~650 ns per dma_start so issue stays far ahead of the ~42 us
            # stream). First weight bytes first; xt/eye are tiny and land
            # before the first matmul needs them. Chunks 0, 9, 10 are
            # split so consuming matmuls start per-slice (Tile tracks
            # range-level DMA->reader dependencies).
            nc.sync.dma_start(wt[0][:, 0:U_OFF // 2],
                              w_d.ap()[0][:, 0:U_OFF // 2])
            nc.sync.dma_start(xt[:], xt_d.ap())
            nc.sync.dma_start(eye[:], eye_d.ap())
            nc.sync.dma_start(wt[0][:, U_OFF // 2:U_OFF],
                              w_d.ap()[0][:, U_OFF // 2:U_OFF])
            nc.sync.dma_start(wt[0][:, U_OFF:WCOLS],
                              w_d.ap()[0][:, U_OFF:WCOLS])
            for c in range(1, NFULL - 1):
                nc.sync.dma_start(wt[c][:], w_d.ap()[c])
            c = NFULL - 1  # chunk 9: gate/down then up
            nc.sync.dma_start(wt[c][:, 0:U_OFF], w_d.ap()[c][:, 0:U_OFF])
            nc.sync.dma_start(wt[c][:, U_OFF:WCOLS],
                              w_d.ap()[c][:, U_OFF:WCOLS])
            # chunk 10 (split layout): half h gd at [h*2048,(h+1)*2048),
            # half h up at [4096+h*1024, 4096+(h+1)*1024)
            nc.sync.dma_start(wt[10][:, 0:2048], w_d.ap()[10][:, 0:2048])
            nc.sync.dma_start(wt[10][:, U_OFF:U_OFF + HIDDEN],
                              w_d.ap()[10][:, U_OFF:U_OFF + HIDDEN])
            nc.sync.dma_start(wt[10][:, 2048:U_OFF],
                              w_d.ap()[10][:, 2048:U_OFF])
            nc.sync.dma_start(wt[10][:, U_OFF + HIDDEN:WCOLS],
                              w_d.ap()[10][:, U_OFF + HIDDEN:WCOLS])

            psum_out = pop.tile([T, HIDDEN], f32)
            itall = itp.tile([P, N_UP * T], f16)
            up_count = [0, 0]    # per-jb position in the accumulation chain
            pending = []         # up-matmul thunks deferred from prev chunk

            def queue_up(c, kidx, upbase):
                it = itall[:, kidx * T:(kidx + 1) * T]
                for jb in range(2):
                    def mm(it=it, c=c, jb=jb, upbase=upbase):
                        k = up_count[jb]
                        up_count[jb] += 1
                        nc.tensor.matmul(
                            psum_out[:, jb * 512:(jb + 1) * 512], it,
                            wt[c][:, upbase + jb * 512:upbase + (jb + 1) * 512],
                            start=(k == 0), stop=(k == N_UP - 1),
                        )
                    pending.append(mm)

            def transpose_strip(inter_slice, kidx):
                tp = tpp.tile([P, T], f32)
                nc.tensor.matmul(tp[:], inter_slice, eye[:])
                nc.vector.tensor_copy(itall[:, kidx * T:(kidx + 1) * T], tp[:])

            for c in range(NFULL):
                base = 32 if c % 2 == 0 else 64
                pgd = pgp.tile([P, 2 * CW], f32)
                todo = pending
                pending = []
                for hc in range(HC):
                    nc.tensor.matmul(
                        pgd[base:base + T, :], xt[:, hc * T:(hc + 1) * T],
                        wt[c][:, hc * 2 * CW:(hc + 1) * 2 * CW],
                        start=(hc == 0), stop=(hc == HC - 1),
                    )
                    if todo and hc % 2 == 1:
                        todo.pop(0)()
                while todo:
                    todo.pop(0)()
                s1 = ip.tile([T, CW], f32)
                nc.scalar.activation(s1[:], pgd[base:base + T, 0:CW],
                                     mybir.ActivationFunctionType.Silu)
                inter = ip.tile([T, CW], f16)
                nc.vector.tensor_mul(inter[:], s1[:],
                                     pgd[base:base + T, CW:2 * CW])
                for f in range(CW // P):
                    kidx = 2 * c + f
                    transpose_strip(inter[:, f * P:(f + 1) * P], kidx)
                    queue_up(c, kidx, U_OFF + f * HIDDEN)

            # chunk 10: two independent 128-wide halves so the chain behind
            # the last weight byte is as short as possible.
            for h in range(2):
                base = 32 if h == 0 else 64
                pgd = pgp.tile([P, CW], f32)
                todo = pending
                pending = []
                for hc in range(HC):
                    nc.tensor.matmul(
                        pgd[base:base + T, 0:CW], xt[:, hc * T:(hc + 1) * T],
                        wt[10][:, h * 2048 + hc * CW:h * 2048 + (hc + 1) * CW],
                        start=(hc == 0), stop=(hc == HC - 1),
                    )
                    if todo and hc % 2 == 1:
                        todo.pop(0)()
                while todo:
                    todo.pop(0)()
                s1 = ip.tile([T, P], f32)
                nc.scalar.activation(s1[:], pgd[base:base + T, 0:P],
                                     mybir.ActivationFunctionType.Silu)
                inter = ip.tile([T, P], f16)
                nc.vector.tensor_mul(inter[:], s1[:],
                                     pgd[base:base + T, P:2 * P])
                kidx = 2 * NFULL + h
                transpose_strip(inter[:], kidx)
                if h == 0:
                    queue_up(10, kidx, U_OFF)
                else:
                    # final strip: emit immediately, jb0 then jb1, so the
                    # jb0 copy (DVE) overlaps the jb1 matmul.
                    it = itall[:, kidx * T:(kidx + 1) * T]
                    for jb in range(2):
                        k = up_count[jb]
                        up_count[jb] += 1
                        nc.tensor.matmul(
                            psum_out[:, jb * 512:(jb + 1) * 512], it,
                            wt[10][:, U_OFF + HIDDEN + jb * 512:
                                  U_OFF + HIDDEN + (jb + 1) * 512],
                            start=(k == 0), stop=(k == N_UP - 1),
                        )
            assert not pending
            assert up_count == [N_UP, N_UP], up_count

            out_sb = op.tile([T, HIDDEN], f32)
            nc.vector.tensor_copy(out_sb[:, 0:512], psum_out[:, 0:512])
            nc.scalar.activation(out_sb[:, 512:1024], psum_out[:, 512:1024],
                                 mybir.ActivationFunctionType.Copy)
            nc.sync.dma_start(out_d.ap(), out_sb[:])

    nc.compile()
    return nc


def _get_compiled():
    global _COMPILED
    if _COMPILED is None:
        _COMPILED = _build()
    return _COMPILED


def _pack_inputs(x, gate_proj, up_proj, down_proj):
    x = np.ascontiguousarray(x, dtype=np.float32)
    # xt[p, hc*T + t] = x[t, hc*128 + p]
    xt = np.ascontiguousarray(
        x.T.reshape(HC, P, T).transpose(1, 0, 2).reshape(P, HC * T)
    ).astype(np.float16)
    eye = np.eye(T, dtype=np.float16)
    in_maps = []
    for k in range(N_CORES):
        g = np.asarray(gate_proj[k], dtype=np.float32)
        d = np.asarray(down_proj[k], dtype=np.float32)
        u = np.asarray(up_proj[k], dtype=np.float32)
        # Full chunks 0..9: wg4/wd4[c, p, hc, o] = g/d[c*CW+o, hc*128+p],
        # interleaved [g 256 | d 256] per hc block.
        wg4 = g.reshape(NCHUNK, CW, HC, P).transpose(0, 3, 2, 1)
        wd4 = d.reshape(NCHUNK, CW, HC, P).transpose(0, 3, 2, 1)
        wgd = np.concatenate([wg4, wd4], axis=3).reshape(NCHUNK, P, 2 * HC * CW)
        # Wu[c, p, f*HIDDEN + j] = u[j, c*CW + f*128 + p]
        wu = u.reshape(HIDDEN, NCHUNK, CW // P, P).transpose(1, 3, 2, 0).reshape(
            NCHUNK, P, 2 * HIDDEN)
        w = np.concatenate([wgd, wu], axis=2).astype(np.float16)
        # Chunk 10 split layout: half h gd cols [h*2048 + hc*256 + o]:
        #   o<128: g[10*CW + h*128 + o, hc*128+p]
        #   o>=128: d[10*CW + h*128 + (o-128), hc*128+p]
        # (up layout for chunk 10 is identical to the full-chunk layout.)
        c = NCHUNK - 1
        glast = g[c * CW:(c + 1) * CW].reshape(2, P, HC, P).transpose(0, 2, 3, 1)
        dlast = d[c * CW:(c + 1) * CW].reshape(2, P, HC, P).transpose(0, 2, 3, 1)
        # glast[h, hc, p, o]; interleave -> [h, hc, p, {g128|d128}]
        gdlast = np.concatenate([glast, dlast], axis=3)  # [2, HC, P, 256]
        w[c, :, 0:U_OFF] = gdlast.transpose(2, 0, 1, 3).reshape(P, U_OFF)
        in_maps.append({"xt": xt, "eye": eye,
                        "w": np.ascontiguousarray(w)})
    return in_maps


def kernel(x, expert_indices, gate_proj, up_proj, down_proj):
    global LAST_RESULTS
    from concourse.bass_utils import run_bass_kernel_spmd

    nc = _get_compiled()
    in_maps = _pack_inputs(x, gate_proj, up_proj, down_proj)
    res = run_bass_kernel_spmd(nc, in_maps, core_ids=list(range(N_CORES)),
                               trace=TRACE)
    LAST_RESULTS = res

    expert_outs = np.stack([res.results[k]["out"] for k in range(N_CORES)])
    idx = np.asarray(expert_indices).astype(np.int64)  # [T, TOP_K]
    return expert_outs[idx, np.arange(T)[:, None], :].astype(np.float32)
